# revision 7
# baseline (speedup 1.0000x reference)
"""MetaLSTM Trainium2 kernel.

Reference computation (T=32 steps, B=262144 learner params, H=16):
    f_new = [x_t, cS, fS] @ WF + bF          # [B,1]
    i_new = [x_t, cS, iS] @ WI + bI          # [B,1]
    d_new = m*dS - sigmoid(i_new)*g_t
    c_new = sigmoid(f_new)*cS + d_new
returns final (fS[1,1,B,1], iS[1,B,1], cS[1,B,1], dS[1,B,1]).

Sharding: pure data parallel over B across 8 cores (32768 b per core).

The only heavy compute is the per-step dot product x_t[b,:16] @ W[:16] for
both gates.  We pre-permute x on the host so a contiguous HBM->SBUF DMA
lands x_t as [128 partitions = (b_lo:8, h:16), 4096 free = b_hi].  Each
[128, 128] column chunk of that tile is fed to the tensor engine as the
*stationary* operand, with a small block-diagonal [128, 16] weight matrix
(both gates x 8 b_lo lanes) as the *moving* operand:

    psum[m, 8g + b_lo] = sum_{b_lo',h} x[(b_lo',h), m] * W_g[h] * [b_lo'==b_lo]
                       = dot_g[b_local = (128cc + m)*8 + b_lo]

so the dots land in PSUM already b-major ([128 = b_hi, 32 chunks x 2 gates
x 8 b_lo]) and the f32 recurrence on VectorE reads them as strided views.
Sigmoids run on ScalarE.  grads ride along in the x DMA (f32 bitcast into
bf16 columns).  Biases bF/bI are folded into the state inits (f0 = bF/wf_f
=> wf_f*f0 contributes bF at every step; exact, and trivially 0 when bF=0).
The b-permutation is undone on the host after the run.
"""

import os
import numpy as np
import ml_dtypes

import concourse.bacc as bacc
import concourse.bass as bass
import concourse.tile as tile
from concourse import mybir
from concourse.bass_utils import run_bass_kernel_spmd

T, B, H = 32, 262144, 16
NCORES = 8
BC = B // NCORES          # 32768 b per core
NB = BC // 8              # 4096 = free dim of x tile
F32 = mybir.dt.float32
BF16 = mybir.dt.bfloat16
AF = mybir.ActivationFunctionType
OP = mybir.AluOpType

_cache = {}


def build_program(reps=1):
    """reps>1 repeats the whole T-step loop back-to-back inside one NEFF
    (same data, states keep evolving) — used only for delta-timing."""
    key = ("nc", reps)
    if key in _cache:
        return _cache[key]
    nc = bacc.Bacc("TRN2", target_bir_lowering=False, debug=False)

    xg_d = nc.dram_tensor("xg", [T, 128, NB + 512], BF16, kind="ExternalInput")
    c0_d = nc.dram_tensor("c0", [128, 256], F32, kind="ExternalInput")
    fi0_d = nc.dram_tensor("fi0", [128, 512], F32, kind="ExternalInput")
    wblk_d = nc.dram_tensor("wblk", [128, 16], BF16, kind="ExternalInput")
    scal_d = nc.dram_tensor("scal", [128, 8], F32, kind="ExternalInput")
    fo_d = nc.dram_tensor("fo", [128, 256], F32, kind="ExternalOutput")
    io_d = nc.dram_tensor("io", [128, 256], F32, kind="ExternalOutput")
    co_d = nc.dram_tensor("co", [128, 256], F32, kind="ExternalOutput")
    do_d = nc.dram_tensor("do", [128, 256], F32, kind="ExternalOutput")

    with tile.TileContext(nc) as tc:
        with (
            tc.tile_pool(name="const", bufs=1) as cpool,
            tc.tile_pool(name="state", bufs=1) as spool,
            tc.tile_pool(name="x", bufs=4) as xpool,
            tc.tile_pool(name="tmp", bufs=3) as tpool,
            tc.tile_pool(name="mm", bufs=2, space=bass.MemorySpace.PSUM) as mpool,
        ):
            wblk = cpool.tile([128, 16], BF16)
            nc.sync.dma_start(wblk[:], wblk_d.ap())
            scal = cpool.tile([128, 8], F32)
            nc.sync.dma_start(scal[:], scal_d.ap())

            fS = spool.tile([128, 256], F32, tag="fS")
            iS = spool.tile([128, 256], F32, tag="iS")
            cS = spool.tile([128, 256], F32, tag="cS")
            dS = spool.tile([128, 256], F32, tag="dS")
            nc.sync.dma_start(fS[:], fi0_d.ap()[:, 0:256])
            nc.sync.dma_start(iS[:], fi0_d.ap()[:, 256:512])
            nc.sync.dma_start(cS[:], c0_d.ap())
            nc.vector.memset(dS[:], 0.0)

            wf_c = scal[:, 0:1]
            wf_f = scal[:, 1:2]
            wi_c = scal[:, 2:3]
            wi_i = scal[:, 3:4]
            m_sc = scal[:, 6:7]

            for t in range(T * reps):
                t = t % T
                xt = xpool.tile([128, NB + 512], BF16)
                nc.sync.dma_start(xt[:], xg_d.ap()[t])
                gt = xt[:, NB : NB + 512].bitcast(F32)  # [128, 256] f32

                dots = mpool.tile([128, 512], F32)
                for cc in range(32):
                    nc.tensor.matmul(
                        dots[:, 16 * cc : 16 * (cc + 1)],
                        xt[:, 128 * cc : 128 * (cc + 1)],
                        wblk[:],
                        start=True,
                        stop=True,
                    )
                # dots[p, 16cc + 8g + b] = dot_g[b_local = 1024cc + 8p + b]
                dv = dots[:].rearrange("p (cc g b) -> p cc g b", cc=32, g=2, b=8)
                fdot = dv[:, :, 0, :]  # [128, 32, 8]
                idot = dv[:, :, 1, :]

                # state free layout: n = 8cc + b
                def v3(ap):
                    return ap.rearrange("p (cc b) -> p cc b", cc=32, b=8)

                s_f = tpool.tile([128, 256], F32, tag="s_f")
                s_i = tpool.tile([128, 256], F32, tag="s_i")
                sg_f = tpool.tile([128, 256], F32, tag="sg_f")
                sg_i = tpool.tile([128, 256], F32, tag="sg_i")
                t3 = tpool.tile([128, 256], F32, tag="t3")
                t4 = tpool.tile([128, 256], F32, tag="t4")

                # s_f = cS*wf_c + fdot ; fS = fS*wf_f + s_f  (fS incl. bias)
                nc.vector.scalar_tensor_tensor(
                    out=v3(s_f[:]), in0=v3(cS[:]), scalar=wf_c, in1=fdot,
                    op0=OP.mult, op1=OP.add,
                )
                nc.vector.scalar_tensor_tensor(
                    out=fS[:], in0=fS[:], scalar=wf_f, in1=s_f[:],
                    op0=OP.mult, op1=OP.add,
                )
                nc.scalar.activation(sg_f[:], fS[:], AF.Sigmoid)

                nc.vector.scalar_tensor_tensor(
                    out=v3(s_i[:]), in0=v3(cS[:]), scalar=wi_c, in1=idot,
                    op0=OP.mult, op1=OP.add,
                )
                nc.vector.scalar_tensor_tensor(
                    out=iS[:], in0=iS[:], scalar=wi_i, in1=s_i[:],
                    op0=OP.mult, op1=OP.add,
                )
                nc.scalar.activation(sg_i[:], iS[:], AF.Sigmoid)

                # dS = dS*m - sg_i*gt ; cS = sg_f*cS + dS
                nc.vector.tensor_tensor(out=t3[:], in0=sg_i[:], in1=gt, op=OP.mult)
                nc.vector.scalar_tensor_tensor(
                    out=dS[:], in0=dS[:], scalar=m_sc, in1=t3[:],
                    op0=OP.mult, op1=OP.subtract,
                )
                nc.vector.tensor_tensor(out=t4[:], in0=sg_f[:], in1=cS[:], op=OP.mult)
                nc.vector.tensor_tensor(out=cS[:], in0=t4[:], in1=dS[:], op=OP.add)

            nc.sync.dma_start(fo_d.ap(), fS[:])
            nc.sync.dma_start(io_d.ap(), iS[:])
            nc.sync.dma_start(co_d.ap(), cS[:])
            nc.sync.dma_start(do_d.ap(), dS[:])

    nc.compile()
    _cache[key] = nc
    return nc


def _safe_div(b, w):
    """b/w, 0 when b==0; used to fold the gate bias into the state init."""
    if b == 0.0:
        return 0.0
    if abs(w) < 1e-30:
        raise NotImplementedError("bias folding needs nonzero recurrent weight")
    return b / w


def _prepare_inputs(x, grads, WF, WI, cI, bI, bF, m):
    """Returns per-core in_maps. All b-permutation happens here."""
    x = np.asarray(x, dtype=np.float32)
    grads = np.asarray(grads, dtype=np.float32).reshape(T, B)
    WF = np.asarray(WF, dtype=np.float32).reshape(H + 2)
    WI = np.asarray(WI, dtype=np.float32).reshape(H + 2)
    cI = np.asarray(cI, dtype=np.float32).reshape(B)
    bI = float(np.asarray(bI).reshape(-1)[0])
    bF = float(np.asarray(bF).reshape(-1)[0])
    m = float(np.asarray(m).reshape(-1)[0])

    # moving weights: col n = 8g + b_lo  ->  W_g[h] on rows (b_lo, h)
    wblk = np.zeros((128, 16), dtype=np.float32)
    for bl in range(8):
        wblk[bl * 16 : (bl + 1) * 16, bl] = WF[:16]
        wblk[bl * 16 : (bl + 1) * 16, 8 + bl] = WI[:16]
    wblk = wblk.astype(ml_dtypes.bfloat16)

    scal = np.zeros((128, 8), dtype=np.float32)
    scal[:, 0] = WF[16]
    scal[:, 1] = WF[17]
    scal[:, 2] = WI[16]
    scal[:, 3] = WI[17]
    scal[:, 4] = bF
    scal[:, 5] = bI
    scal[:, 6] = m

    fi0 = np.zeros((128, 512), dtype=np.float32)
    fi0[:, 0:256] = _safe_div(bF, WF[17])
    fi0[:, 256:512] = _safe_div(bI, WI[17])

    in_maps = []
    for s in range(NCORES):
        sl = slice(s * BC, (s + 1) * BC)
        # x: [T, BC, 16] -> [T, 128=(b_lo,h), NB=b_hi]
        xs = x[:, sl, :].reshape(T, NB, 8, 16).transpose(0, 2, 3, 1).reshape(T, 128, NB)
        xs = np.ascontiguousarray(xs).astype(ml_dtypes.bfloat16)
        # g: b_local = 1024cc + 8p + b  ->  [T, 128, (cc, b)]
        gs = (
            grads[:, sl]
            .reshape(T, 32, 128, 8)
            .transpose(0, 2, 1, 3)
            .reshape(T, 128, 256)
        )
        gs = np.ascontiguousarray(gs)
        xg = np.concatenate(
            [xs.view(np.uint16), gs.view(np.uint16).reshape(T, 128, 512)], axis=2
        ).view(ml_dtypes.bfloat16)
        c0 = cI[sl].reshape(32, 128, 8).transpose(1, 0, 2).reshape(128, 256)
        in_maps.append(
            {
                "xg": np.ascontiguousarray(xg),
                "c0": np.ascontiguousarray(c0),
                "fi0": fi0,
                "wblk": wblk,
                "scal": scal,
            }
        )
    return in_maps


def _unperm(o):
    """[128, 256] core tile -> [BC] in b_local order."""
    return o.reshape(128, 32, 8).transpose(1, 0, 2).reshape(BC)


def kernel(x, grads, WF, WI, cI, bI, bF, m):
    nc = build_program(1)
    in_maps = _prepare_inputs(x, grads, WF, WI, cI, bI, bF, m)
    trace = bool(int(os.environ.get("METALSTM_TRACE", "0")))
    res = run_bass_kernel_spmd(nc, in_maps, list(range(NCORES)), trace=trace)
    _cache["last_result"] = res

    outs = {name: np.empty(B, dtype=np.float32) for name in ("fo", "io", "co", "do")}
    for s in range(NCORES):
        sl = slice(s * BC, (s + 1) * BC)
        for name in outs:
            outs[name][sl] = _unperm(res.results[s][name])

    fS = outs["fo"].reshape(1, 1, B, 1)
    iS = outs["io"].reshape(1, B, 1)
    cS = outs["co"].reshape(1, B, 1)
    dS = outs["do"].reshape(1, B, 1)
    return fS, iS, cS, dS
